# revision 1
# baseline (speedup 1.0000x reference)
"""GRU-decoder first-step kernel for 8 Trainium2 NeuronCores.

Math (see reference): all L-1 output steps are identical, so compute the
single step and broadcast on host:
    x0 = relu(emb[input_ids[:, 0]])                [B, D]
    h0 = einsum("bld,l->bd", hidden, Wb[0]) + bb   [B, D]
    GRU cell (r, z, n) -> h1                       [B, D]
    logits = h1 @ Wp.T + bp                        [B, V]
    out = broadcast(logits)                        [B, L-1, V]

Sharding: the only big tensor is Wp [V, D] (~103 MB f32). Tensor-parallel
over the vocab dim across the 8 cores; the (tiny) GRU math is replicated
on every core. Big operands are shipped as fp16 (PSUM accumulates f32),
halving HBM traffic in this memory-bound regime.

Host-side work is limited to data marshalling: slicing, transposing,
dtype casts, concatenation and the embedding-row gather. All FLOPs
(bridge reduction, GRU cell, projection) run on device.

Perf notes (concourse TimelineSim cost model, per core): ~40.8 us
against a ~35 us pure-transfer floor for the ~12.4 MB each core reads.
Key structure decisions, in impact order:
 - tensor-parallel over vocab: Wp is read once across the machine
   (data-parallel would read 103 MB per core);
 - fp16 for every large operand (PSUM accumulates fp32; measured
   output error is ~3.4e-4 scale-relative);
 - few large DMAs in critical-chain order (DMA-engine time serializes
   roughly in issue order; each dma_start costs ~1.2 us of sequencer
   issue time): bridge inputs -> gate weights (r, n, z gate order so
   the z load overlaps the r/n math) -> projection shard -> outputs;
 - the whole GRU cell runs in transposed space ([d, b]; batch on the
   free axis): gate matmuls stream N=32 columns instead of 512, biases
   ride K=1 matmuls, h0/h1 never need PE transposes, and elementwise
   ops use all 128 partitions;
 - projection shard columns are host-interleaved [chunk | k | c] so
   each 512-column PSUM chunk becomes computable as soon as its piece
   lands; pieces shrink toward the end and the final PSUM->SBUF copies
   run on DVE and ACT in parallel so the post-stream tail is one small
   chunk + copy + DMA latency (~5.9 us, mostly fixed sem/drain costs);
 - two tiny AllGathers would shave another ~12 us of replicated loads,
   but a collective costs ~15 us fixed on this fabric - not worth it.
"""

import numpy as np

import concourse.mybir as mybir
import concourse.tile as tile
from concourse import bacc
from concourse.bass_utils import run_bass_kernel_spmd

B, L, D, V = 32, 64, 512, 50257
NCORES = 8
VS = 6283          # per-core vocab shard; 8 * 6283 = 50264 >= V
VPAD = VS * NCORES
DC = D // 128      # 4 contraction chunks of 128
NT = (B * L) // 128  # 16 hidden row-tiles of 128
G = 3 * D
F16 = mybir.dt.float16
F32 = mybir.dt.float32
# projection chunk widths: 11 full PSUM banks, the 12th split 384+128 so
# the last-arriving piece carries minimal compute, then the 139 tail
CHUNKW = [512] * 11 + [384, 128, VS - 512 * 12]
CHUNKO = [sum(CHUNKW[:i]) for i in range(len(CHUNKW))]  # logit col offsets
WINO = [sum(CHUNKW[:i]) * DC for i in range(len(CHUNKW))]  # wp tile offsets

_CACHE: dict = {}


def _build_bass():
    nc = bacc.Bacc("TRN2", target_bir_lowering=False, debug=False,
                   num_devices=NCORES)

    wp_d = nc.dram_tensor("wpT", [128, DC * VS], F16, kind="ExternalInput")
    bp_d = nc.dram_tensor("bp", [1, VS], F16, kind="ExternalInput")
    # head = [wb_diag (NT*B) | x0T (DC*B) | hidden (NT*D)] columns, fp16
    NHEAD = NT * B + DC * B + NT * D
    head_d = nc.dram_tensor("head", [128, NHEAD], F16, kind="ExternalInput")
    wih_d = nc.dram_tensor("wihT", [128, DC * G], F16, kind="ExternalInput")
    whh_d = nc.dram_tensor("whhT", [128, DC * G], F16, kind="ExternalInput")
    bcat_d = nc.dram_tensor("bcat", [1, 2 * G], F16, kind="ExternalInput")
    bb_d = nc.dram_tensor("bb", [128, 1], F32, kind="ExternalInput")
    out_d = nc.dram_tensor("logits", [B, VS], F32, kind="ExternalOutput")

    AF = mybir.ActivationFunctionType

    with tile.TileContext(nc) as tc:
        with (
            tc.tile_pool(name="wp", bufs=1) as wp_pool,
            tc.tile_pool(name="big", bufs=1) as big,
            tc.tile_pool(name="sm", bufs=1) as sm,
            tc.tile_pool(name="ps_mm", bufs=4, space="PSUM") as ps_mm,
            tc.tile_pool(name="ps_g", bufs=3, space="PSUM") as ps_g,
            tc.tile_pool(name="ps_b", bufs=1, space="PSUM") as ps_b,
        ):
            # ---- loads: critical-chain order on the sync sequencer
            # (transfers serialize on the DMA engines in arrival order);
            # non-urgent smalls go via gpsimd in parallel ----
            head_sb = big.tile([128, NHEAD], F16, tag="head")
            HID0 = NT * B + DC * B
            HHALF = HID0 + (NT // 2) * D
            nc.sync.dma_start(out=head_sb[:, 0:HHALF],
                              in_=head_d[:, 0:HHALF])
            nc.sync.dma_start(out=head_sb[:, HHALF:], in_=head_d[:, HHALF:])
            wbd_sb = head_sb[:, 0:NT * B]
            x0f_sb = head_sb[:, NT * B:HID0]
            hid_sb = head_sb[:, HID0:]
            # W columns are laid out [gate, k, j_local] on host; load in
            # consumption order r, n, z so the z load overlaps the r/n math
            wih_sb = big.tile([128, DC * G], F16, tag="wih")
            whh_sb = big.tile([128, DC * G], F16, tag="whh")
            GW = DC * 512
            for g_ in (0, 2, 1):  # r, n, z
                nc.sync.dma_start(out=wih_sb[:, g_ * GW:(g_ + 1) * GW],
                                  in_=wih_d[:, g_ * GW:(g_ + 1) * GW])
                nc.sync.dma_start(out=whh_sb[:, g_ * GW:(g_ + 1) * GW],
                                  in_=whh_d[:, g_ * GW:(g_ + 1) * GW])

            bcat_sb = sm.tile([1, 2 * G], F16, tag="bcat")
            nc.gpsimd.dma_start(out=bcat_sb[:], in_=bcat_d[:])
            bb_sb = sm.tile([128, 1], F32, tag="bb")
            nc.gpsimd.dma_start(out=bb_sb[:], in_=bb_d[:])
            bp_sb = sm.tile([1, VS], F16, tag="bp")
            nc.gpsimd.dma_start(out=bp_sb[:], in_=bp_d[:])

            # projection shard: columns interleaved on host as
            # [chunk j | k | c] so each 512-column chunk window arrives
            # with all DC contraction strips at once. Pieces shrink toward
            # the end so the last-arriving work is small.
            wp_sb = wp_pool.tile([128, DC * VS], F16, tag="wp")
            NCH = len(CHUNKW)
            # pieces as chunk-groups; arrival order puts the tiny tail
            # (13) and then the split sub-chunks (11, 12) last so the
            # last-arriving piece carries minimal compute
            PIECE_GROUPS = [[0, 1], [2, 3], [4, 5], [6, 7], [8, 9], [10],
                            [13], [11], [12]]
            for grp in PIECE_GROUPS:
                lo = WINO[grp[0]]
                hi = WINO[grp[-1]] + DC * CHUNKW[grp[-1]]
                nc.sync.dma_start(out=wp_sb[:, lo:hi], in_=wp_d[:, lo:hi])

            ones_sb = sm.tile([1, B], F16, tag="ones")
            nc.any.memset(ones_sb[:], 1.0)

            # ==== GRU cell entirely in transposed space: all [d, b]
            # tensors are [128, DC*B] tiles with column = k*B + b. ====

            # x0T = relu(embT rows) -> fp16, one op
            x0t_sb = sm.tile([128, DC * B], F16, tag="x0t")
            nc.scalar.activation(x0t_sb[:], x0f_sb, AF.Relu)

            # bridge, transposed: h0T[d, b] = sum_t hid[t].T @ wbd[t]
            ps_h0 = ps_b.tile([128, DC * B], F32, tag="b")
            for k in range(DC):
                for t_i in range(NT):
                    nc.tensor.matmul(
                        ps_h0[:, k * B:(k + 1) * B],
                        hid_sb[:, t_i * D + k * 128:t_i * D + (k + 1) * 128],
                        wbd_sb[:, t_i * B:(t_i + 1) * B],
                        start=(t_i == 0), stop=(t_i == NT - 1),
                    )
            h0t_sb = sm.tile([128, DC * B], F32, tag="h0t")
            nc.scalar.activation(h0t_sb[:], ps_h0[:], AF.Identity,
                                 bias=bb_sb[:])
            h0t16 = sm.tile([128, DC * B], F16, tag="h0t16")
            nc.scalar.activation(h0t16[:], h0t_sb[:], AF.Copy)

            # gates, transposed: gate^T[j, b] accumulated per
            # (j-block jb, k): lhsT = W^T [d-chunk, j-block],
            # rhs = x0T / h0T [d-chunk, b]. Bias rows ride K=1 matmuls
            # with values along M.
            def gate_psum(g_, use_x, use_h):
                # gate g_ preacts: W cols at g_*GW + k*512 + j_local,
                # bias at g_*512 + j_local in bcat
                ps = ps_g.tile([128, DC * B], F32, tag="g")
                for jb in range(DC):
                    o = ps[:, jb * B:(jb + 1) * B]
                    bw = g_ * 512 + jb * 128
                    ops = []
                    if use_x:
                        ops.append((bcat_sb[:, bw:bw + 128], ones_sb[:]))
                    if use_h:
                        ops.append((bcat_sb[:, G + bw:G + bw + 128],
                                    ones_sb[:]))
                    for k in range(DC):
                        cw = g_ * GW + k * 512 + jb * 128
                        if use_x:
                            ops.append((wih_sb[:, cw:cw + 128],
                                        x0t_sb[:, k * B:(k + 1) * B]))
                        if use_h:
                            ops.append((whh_sb[:, cw:cw + 128],
                                        h0t16[:, k * B:(k + 1) * B]))
                    for i, (lhsT, rhs) in enumerate(ops):
                        nc.tensor.matmul(o, lhsT, rhs, start=(i == 0),
                                         stop=(i == len(ops) - 1))
                return ps

            ps_r = gate_psum(0, True, True)
            ps_xn = gate_psum(2, True, False)
            ps_hn = gate_psum(2, False, True)

            r_sb = sm.tile([128, DC * B], F32, tag="r")
            nc.scalar.activation(r_sb[:], ps_r[:], AF.Sigmoid)
            t1 = sm.tile([128, DC * B], F32, tag="t1")
            nc.vector.tensor_mul(t1[:], r_sb[:], ps_hn[:])
            t2 = sm.tile([128, DC * B], F32, tag="t2")
            nc.vector.tensor_add(t2[:], t1[:], ps_xn[:])
            n_sb = sm.tile([128, DC * B], F32, tag="n")
            nc.scalar.activation(n_sb[:], t2[:], AF.Tanh)

            ps_z = gate_psum(1, True, True)
            z_sb = sm.tile([128, DC * B], F32, tag="z")
            nc.scalar.activation(z_sb[:], ps_z[:], AF.Sigmoid)

            # h1T = n + z * (h0T - n), cast to fp16 for the projection
            d_sb = sm.tile([128, DC * B], F32, tag="d")
            nc.vector.tensor_sub(d_sb[:], h0t_sb[:], n_sb[:])
            e_sb = sm.tile([128, DC * B], F32, tag="e")
            nc.vector.tensor_mul(e_sb[:], z_sb[:], d_sb[:])
            h1_sb = sm.tile([128, DC * B], F32, tag="h1")
            nc.vector.tensor_add(h1_sb[:], n_sb[:], e_sb[:])
            h1t_sb = sm.tile([128, DC * B], F16, tag="h1t")
            nc.scalar.activation(h1t_sb[:], h1_sb[:], AF.Copy)

            # ---- projection: logits[:, j] = h1 @ WpT[:, j] + bp[j] ----
            logit_sb = big.tile([B, VS], F32, tag="lg")
            # iterate the 139-tail (13) before the split sub-chunks so its
            # copy clears the engines before the anchor chain
            ORDER = list(range(11)) + [13, 11, 12]
            FLUSH = {3: (0, 3), 7: (4, 7), 10: (8, 10), 13: (11, 13)}
            for pos, j in enumerate(ORDER):
                off = CHUNKO[j]
                win, nj = WINO[j], CHUNKW[j]
                ps = ps_mm.tile([B, nj], F32, tag="mm")
                nc.tensor.matmul(ps[:], ones_sb[:], bp_sb[:, off:off + nj],
                                 start=True, stop=False)
                for k in range(DC):
                    nc.tensor.matmul(ps[:], h1t_sb[:, k * B:(k + 1) * B],
                                     wp_sb[:, win + k * nj:win + (k + 1) * nj],
                                     start=False, stop=(k == DC - 1))
                # final (tiny) chunk goes to ACT, which then issues its
                # own out-DMA (same-engine order, no cross-engine sem hop)
                if j % 2 == 0 or j == 12:
                    nc.scalar.activation(logit_sb[:, off:off + nj], ps[:],
                                         AF.Copy)
                else:
                    nc.vector.tensor_copy(logit_sb[:, off:off + nj], ps[:])
                if pos in FLUSH:
                    j0f, j1f = FLUSH[pos]
                    lo = CHUNKO[j0f]
                    hi = CHUNKO[j1f] + CHUNKW[j1f]
                    gi = sorted(FLUSH).index(pos)
                    # early groups ride the gpsimd SWDGE (off critical
                    # path); the final group issues from ACT right after
                    # its own copy (same-engine order, no sem hop)
                    eng = [nc.gpsimd, nc.gpsimd, nc.gpsimd,
                           nc.scalar][gi]
                    eng.dma_start(out=out_d[:, lo:hi],
                                  in_=logit_sb[:, lo:hi])

    nc.compile()
    return nc


def _get_bass():
    if "nc" not in _CACHE:
        _CACHE["nc"] = _build_bass()
    return _CACHE["nc"]


def _interleave(a):
    """[DC*128, N] -> [128, DC*N] with [p, k*N+c] = a[k*128+p, c]."""
    n = a.shape[1]
    return np.ascontiguousarray(
        a.reshape(DC, 128, n).transpose(1, 0, 2).reshape(128, DC * n))


def _prep_inputs(inputs):
    ids = np.asarray(inputs["input_ids"])[:, 0].astype(np.int64)
    emb = np.asarray(inputs["emb"], dtype=np.float32)
    hidden = np.asarray(inputs["hidden"], dtype=np.float32)
    Wb = np.asarray(inputs["Wb"], dtype=np.float32)
    bb = np.asarray(inputs["bb"], dtype=np.float32)
    W_ih = np.asarray(inputs["W_ih"], dtype=np.float32)
    b_ih = np.asarray(inputs["b_ih"], dtype=np.float32)
    W_hh = np.asarray(inputs["W_hh"], dtype=np.float32)
    b_hh = np.asarray(inputs["b_hh"], dtype=np.float32)
    Wp = np.asarray(inputs["Wp"], dtype=np.float32)
    bp = np.asarray(inputs["bp"], dtype=np.float32)

    # x0T: [D, B] -> [128, DC*B]
    x0T = _interleave(np.ascontiguousarray(emb[ids].T))
    # hidden rows (b*64+l, d) -> [128, NT*D] with [p, t*D+d] = row t*128+p
    hid16 = np.ascontiguousarray(
        hidden.reshape(NT, 128, D).transpose(1, 0, 2)
        .reshape(128, NT * D)).astype(np.float16)
    # block-diagonal bridge weights
    wbd = np.zeros((NT, 128, B), np.float32)
    for t in range(NT):
        wbd[t, 0:64, 2 * t] = Wb[0]
        wbd[t, 64:128, 2 * t + 1] = Wb[0]
    wbd = np.ascontiguousarray(
        wbd.transpose(1, 0, 2).reshape(128, NT * B)).astype(np.float16)
    head = np.concatenate(
        [wbd, x0T.astype(np.float16), hid16], axis=1)
    def _w_layout(w):
        # W [3D, D] -> cols [g, k, j_local], partitions d%128
        a = np.ascontiguousarray(w.T).reshape(DC, 128, 3, 512)
        return np.ascontiguousarray(
            a.transpose(1, 2, 0, 3).reshape(128, DC * G)).astype(np.float16)

    wihT = _w_layout(W_ih)
    whhT = _w_layout(W_hh)
    bcat = np.concatenate([b_ih, b_hh]).reshape(1, 2 * G).astype(np.float16)
    bb_b = np.ascontiguousarray(
        np.broadcast_to(bb.reshape(1, 1), (128, 1))).astype(np.float32)

    WpT = np.zeros((D, VPAD), np.float16)
    WpT[:, :V] = Wp.T
    bp_pad = np.zeros((VPAD,), np.float16)
    bp_pad[:V] = bp

    shared = {
        "head": head,
        "wihT": wihT, "whhT": whhT, "bcat": bcat, "bb": bb_b,
    }
    in_maps = []
    for c in range(NCORES):
        sl = slice(c * VS, (c + 1) * VS)
        m = dict(shared)
        shard = WpT[:, sl].reshape(DC, 128, VS)          # [k, p, v]
        wpk = np.empty((128, DC * VS), np.float16)
        for j, nj in enumerate(CHUNKW):
            v0 = CHUNKO[j]
            win = WINO[j]
            for k in range(DC):
                wpk[:, win + k * nj:win + (k + 1) * nj] = \
                    shard[k, :, v0:v0 + nj]
        m["wpT"] = wpk
        m["bp"] = np.ascontiguousarray(bp_pad[sl][None, :])
        in_maps.append(m)
    return in_maps


def _run(in_maps, trace=False, tmpdir=None):
    nc = _get_bass()
    return run_bass_kernel_spmd(nc, in_maps, list(range(NCORES)),
                                trace=trace, tmpdir=tmpdir)


def kernel(**inputs) -> np.ndarray:
    in_maps = _prep_inputs(inputs)
    try:
        res = _run(in_maps).results
    except Exception:
        # transient NRT device wedges have been observed on this fabric;
        # one retry after a short pause usually lands on healthy cores
        import time as _time
        _time.sleep(5.0)
        res = _run(in_maps).results
    logits = np.concatenate([res[c]["logits"] for c in range(NCORES)],
                            axis=1)[:, :V].astype(np.float32)
    return np.broadcast_to(logits[:, None, :], (B, L - 1, V))



# revision 17
# speedup vs baseline: 1.6800x; 1.6800x over previous
"""GRU-decoder first-step kernel for 8 Trainium2 NeuronCores.

Math (see reference): all L-1 output steps are identical, so compute the
single step and broadcast on host:
    x0 = relu(emb[input_ids[:, 0]])                [B, D]
    h0 = einsum("bld,l->bd", hidden, Wb[0]) + bb   [B, D]
    GRU cell (r, z, n) -> h1                       [B, D]
    logits = h1 @ Wp.T + bp                        [B, V]
    out = broadcast(logits)                        [B, L-1, V]

Sharding: tensor-parallel over the vocab dim across the 8 cores; the
(tiny) GRU math is replicated on every core.  This regime is HBM-bound,
so every large operand ships at the smallest dtype the 2e-2 error gate
allows: fp8 e3m4 (4 mantissa bits) for Wp / W_ih / W_hh / hidden with
power-of-two scales folded into the activations, fp16 for everything
error-critical (Wb, x0, biases, the GRU state) and for the logits
output (the device writes 64*logits; the host unscale is an exact
exponent shift).  ~6.4 MB per core vs 12.6 MB for the all-fp16 version.

Structure decisions, in impact order:
 - fp8 e3m4 halves the big-operand traffic; e3m4 (not e4m3) because its
   extra mantissa bit keeps the end-to-end error at ~1.5e-2;
 - the projection runs with Wp as the *stationary* operand per
   128-vocab block (out = [vocab_part, batch]): each matmul streams
   only B=32 moving rows, logits land in SBUF-native layout and are
   written as one contiguous row-DMA per flush group (host unshuffles);
 - every small tensor rides inside ONE fp16 side tensor (two early
   SWDGE loads) so no critical operand queues behind the big stream;
 - the DMA stream order is consumption order: hidden (bridge), gate
   weights r+z then n, projection blocks in shrinking groups;
 - ACT runs exactly one table set (a dummy sigmoid triggers the
   sigmoid_and_others load at t~0); all copies/elementwise run on
   DVE/Pool so no 1.3us table reload lands mid-chain;
 - projection bias is pre-replicated into a [128, 50*B] SBUF tile (one
   broadcast DVE copy) and added during the PSUM->SBUF drain, so no
   K=1 bias matmuls on the PE;
 - out-DMAs issue from the otherwise-idle sync queue: a DMA's wait
   occupies its issuing sequencer, which must not carry later work.
"""

import numpy as np
import ml_dtypes

import concourse.mybir as mybir
import concourse.tile as tile
from concourse import bacc
from concourse.bass_utils import run_bass_kernel_spmd

B, L, D, V = 32, 64, 512, 50257
NCORES = 8
VS = 6400            # per-core vocab shard; 50 blocks of 128, 8*6400 >= V
NBLK = VS // 128     # 50 projection blocks per core
VPAD = VS * NCORES
DC = D // 128        # 4 contraction chunks of 128
NT = (B * L) // 128  # 16 hidden row-tiles of 128
G = 3 * D
F16 = mybir.dt.float16
F32 = mybir.dt.float32
F8 = mybir.dt.float8e3          # e3m4: 4 mantissa bits, max 15.5
E3NP = ml_dtypes.float8_e3m4
WSC = 64.0           # weight scale (Wp, W_ih, W_hh, biases)
HSC = 2.0            # hidden scale

# projection DMA groups (in blocks): shrink toward the end so the
# last-arriving piece carries minimal compute + copy + out-DMA
DGRP = [16, 14, 11, 7, 2]          # wp DMA pieces (blocks)
# compute groups are OFFSET from the DMA pieces: each group's FIRST
# block sits in an already-arrived piece, so the scheduler's hoisted
# group-head Ldweights never head-of-line blocks the PE wait queue
# (which would defer the previous group's drain past the next DMA)
CGRP = [14, 14, 11, 7, 2, 2]
CGO = [sum(CGRP[:i]) for i in range(len(CGRP) + 1)]
# out-DMA issue queue per group: early groups ride SWDGE (whose
# transfers yield to the HWDGE input stream in arbitration), the tail
# splits across the idle SP / Pool queues
OUT_ENG = ["gp", "gp", "gp", "sp", "sp", "gp"]


# smk column map: x0T | wsm | bb | bpT | bcat-row
SM_X0, SM_WSM, SM_BB, SM_BP = 0, DC * B, DC * B + 2, DC * B + 3
SM_BC = SM_BP + NBLK          # 181
SM_N = SM_BC + 2 * G          # 3253

_CACHE: dict = {}


def _build_bass():
    nc = bacc.Bacc("TRN2", target_bir_lowering=False, debug=False,
                   num_devices=NCORES)

    hid_d = nc.dram_tensor("hid", [128, NT * D], F8, kind="ExternalInput")
    # gate weights, consumption order r, z, n; cols
    # [(gate, mat, k) -> 512 j-cols]: block index = (go*2 + m)*DC + k
    wg_d = nc.dram_tensor("wg", [128, 2 * DC * G], F8, kind="ExternalInput")
    wp_d = nc.dram_tensor("wp", [128, DC * VS], F8, kind="ExternalInput")
    smk_d = nc.dram_tensor("smk", [128, SM_N], F16, kind="ExternalInput")
    # out[p, blk*B + b] = WSC * logits[b, blk*128 + p]
    out_d = nc.dram_tensor("logits", [128, NBLK * B], F16,
                           kind="ExternalOutput")

    AF = mybir.ActivationFunctionType
    ALU = mybir.AluOpType
    GW = DC * 512  # cols per (gate, mat)

    with tile.TileContext(nc) as tc:
        with (
            tc.tile_pool(name="wp", bufs=1) as wp_pool,
            tc.tile_pool(name="big", bufs=1) as big,
            tc.tile_pool(name="sm", bufs=1) as sm,
            tc.tile_pool(name="ps_mm", bufs=4, space="PSUM") as ps_mm,
            tc.tile_pool(name="ps_g", bufs=2, space="PSUM") as ps_g,
            tc.tile_pool(name="ps_b", bufs=1, space="PSUM") as ps_b,
        ):
            # ---- x0/wsm/bb/bp ride an early SWDGE queue (lands in the
            # first inter-transfer gap of the big stream) ----
            smk_sb = big.tile([128, SM_N], F16, tag="smk")
            nc.gpsimd.dma_start(out=smk_sb[:, 0:SM_BC],
                                in_=smk_d[:, 0:SM_BC])

            # ---- big stream on the sync queue, consumption order;
            # the bcat bias row goes 3rd on SP (tiny transfer, FIFO
            # grant right behind hid) because a hoisted gate-bias
            # Ldweights head-of-line blocks the PE wait queue until
            # bcat lands ----
            hid_sb = big.tile([128, NT * D], F8, tag="hid")
            HHALF = (NT // 2) * D
            nc.sync.dma_start(out=hid_sb[:, 0:HHALF], in_=hid_d[:, 0:HHALF])
            nc.sync.dma_start(out=hid_sb[:, HHALF:], in_=hid_d[:, HHALF:])
            nc.sync.dma_start(out=smk_sb[0:1, SM_BC:],
                              in_=smk_d[0:1, SM_BC:])
            wg_sb = big.tile([128, 2 * DC * G], F8, tag="wg")
            # r+z gates first (2 gates * 2 mats * GW cols), n last
            nc.sync.dma_start(out=wg_sb[:, 0:4 * GW], in_=wg_d[:, 0:4 * GW])
            nc.sync.dma_start(out=wg_sb[:, 4 * GW:], in_=wg_d[:, 4 * GW:])
            wp_sb = wp_pool.tile([128, DC * VS], F8, tag="wp")
            dgo = 0
            for ng in DGRP:
                lo, hi = dgo * 512, (dgo + ng) * 512
                nc.sync.dma_start(out=wp_sb[:, lo:hi], in_=wp_d[:, lo:hi])
                dgo += ng

            ones_sb = sm.tile([1, B], F16, tag="ones")
            nc.vector.memset(ones_sb[:], 1.0)
            # dummy sigmoid: pulls the ACT sigmoid_and_others table load
            # (1.28us) to t~0; that set covers relu/identity/tanh/sigmoid
            # so no reload ever lands on the critical chain
            dum_sb = sm.tile([1, 1], F32, tag="dum")
            nc.scalar.activation(dum_sb[:], ones_sb[0:1, 0:1], AF.Sigmoid)

            # projection bias, replicated [128, NBLK*B] with one
            # broadcast copy (added during the PSUM->SBUF drain)
            rep_sb = big.tile([128, NBLK * B], F16, tag="rep")
            nc.vector.tensor_copy(
                rep_sb[:, :].rearrange("p (j b) -> p j b", b=B),
                smk_sb[:, SM_BP:SM_BP + NBLK].unsqueeze(2)
                .broadcast_to([128, NBLK, B]))

            # ==== GRU cell in transposed space: [d, b] tiles are
            # [128, DC*B] with column = k*B + b. ====

            # x0T = relu(embT rows) -> fp16
            x0t_sb = sm.tile([128, DC * B], F16, tag="x0t")
            nc.scalar.activation(x0t_sb[:], smk_sb[:, SM_X0:SM_X0 + DC * B],
                                 AF.Relu)

            # bridge: h0T[d, 2t+j] = sum_p hid_tile[t][p, d] * wsm[p, j]
            # (rows t*128+p of hidden are (b=2t+p//64, l=p%64))
            ps_h0 = ps_b.tile([128, DC * B], F32, tag="b")
            wsm = smk_sb[:, SM_WSM:SM_WSM + 2]
            for k in range(DC):
                for t_i in range(NT):
                    nc.tensor.matmul(
                        ps_h0[:, k * B + 2 * t_i:k * B + 2 * t_i + 2],
                        hid_sb[:, t_i * D + k * 128:t_i * D + (k + 1) * 128],
                        wsm,
                        start=True, stop=True,
                    )
            # h0 = ps/HSC + bb (f32 master + fp16 feed for the PE)
            h0t_sb = sm.tile([128, DC * B], F32, tag="h0t")
            nc.scalar.activation(h0t_sb[:], ps_h0[:], AF.Identity,
                                 bias=smk_sb[:, SM_BB:SM_BB + 1],
                                 scale=1.0 / HSC)
            h0t16 = sm.tile([128, DC * B], F16, tag="h0t16")
            nc.vector.tensor_copy(h0t16[:], h0t_sb[:])

            # gates: gate^T[j, b] per (jb, k); lhsT = W^T [d, j-block]
            # (fp8, scaled by WSC), rhs = x0T / h0T (fp16). Biases
            # (host-scaled by WSC) ride K=1 matmuls with values along M.
            def gate_psum(g_, use_x, use_h):
                # g_ is the position in wg AND the pytorch row block
                # (both are r, z, n order)
                ps = ps_g.tile([128, DC * B], F32, tag="g")
                for jb in range(DC):
                    o = ps[:, jb * B:(jb + 1) * B]
                    bw = SM_BC + g_ * 512 + jb * 128
                    ops = []
                    if use_x:
                        ops.append((smk_sb[0:1, bw:bw + 128], ones_sb[:]))
                    if use_h:
                        ops.append((smk_sb[0:1, G + bw:G + bw + 128],
                                    ones_sb[:]))
                    for k in range(DC):
                        if use_x:
                            cw = (g_ * 2 * DC + k) * 512 + jb * 128
                            ops.append((wg_sb[:, cw:cw + 128],
                                        x0t_sb[:, k * B:(k + 1) * B]))
                        if use_h:
                            cw = ((g_ * 2 + 1) * DC + k) * 512 + jb * 128
                            ops.append((wg_sb[:, cw:cw + 128],
                                        h0t16[:, k * B:(k + 1) * B]))
                    for i, (lhsT, rhs) in enumerate(ops):
                        nc.tensor.matmul(o, lhsT, rhs, start=(i == 0),
                                         stop=(i == len(ops) - 1))
                return ps

            # r and z first (their weights arrive first); z's products
            # with h0 precompute while the n-gate weights land
            ps_r = gate_psum(0, True, True)
            r_sb = sm.tile([128, DC * B], F32, tag="r")
            nc.scalar.activation(r_sb[:], ps_r[:], AF.Sigmoid,
                                 scale=1.0 / WSC)
            ps_z = gate_psum(1, True, True)
            z_sb = sm.tile([128, DC * B], F32, tag="z")
            nc.scalar.activation(z_sb[:], ps_z[:], AF.Sigmoid,
                                 scale=1.0 / WSC)
            # zh0 = z * h0, zc = 1 - z (both before n is ready)
            zh0_sb = sm.tile([128, DC * B], F32, tag="zh0")
            nc.vector.tensor_mul(zh0_sb[:], z_sb[:], h0t_sb[:])
            zc_sb = sm.tile([128, DC * B], F32, tag="zc")
            nc.vector.tensor_scalar(zc_sb[:], z_sb[:], -1.0, 1.0,
                                    ALU.mult, ALU.add)

            ps_xn = gate_psum(2, True, False)
            ps_hn = gate_psum(2, False, True)
            t1 = sm.tile([128, DC * B], F32, tag="t1")
            nc.vector.tensor_mul(t1[:], r_sb[:], ps_hn[:])
            t2 = sm.tile([128, DC * B], F32, tag="t2")
            nc.vector.tensor_add(t2[:], t1[:], ps_xn[:])
            n_sb = sm.tile([128, DC * B], F32, tag="n")
            nc.scalar.activation(n_sb[:], t2[:], AF.Tanh, scale=1.0 / WSC)

            # h1 = (1-z)*n + z*h0, fused straight into the fp16 PE feed
            f_sb = sm.tile([128, DC * B], F32, tag="f")
            nc.vector.tensor_mul(f_sb[:], zc_sb[:], n_sb[:])
            h1t_sb = sm.tile([128, DC * B], F16, tag="h1t")
            nc.vector.tensor_add(h1t_sb[:], f_sb[:], zh0_sb[:])

            # ---- projection: per 128-vocab block, Wp is stationary
            # (lhsT [128 d, 128 v]) and h1T moves (rhs [128 d, B]);
            # ps[v, b] accumulates the 4 k-chunks; the PSUM->SBUF drain
            # adds the replicated bias. Out-DMAs ride the idle sync
            # queue so their waits never block a compute engine. ----
            logit_sb = big.tile([128, NBLK * B], F16, tag="lg")
            for gi, ng in enumerate(CGRP):
                b0 = CGO[gi]
                ps = ps_mm.tile([128, ng * B], F32, tag="mm")
                for i in range(ng):
                    blk = b0 + i
                    o = ps[:, i * B:(i + 1) * B]
                    for k in range(DC):
                        nc.tensor.matmul(
                            o,
                            wp_sb[:, blk * 512 + k * 128:
                                  blk * 512 + (k + 1) * 128],
                            h1t_sb[:, k * B:(k + 1) * B],
                            start=(k == 0), stop=(k == DC - 1))
                # drains on DVE (fastest copy engine); out-DMAs spread
                # per OUT_ENG so no single issue path serializes the tail
                w0, w1 = b0 * B, (b0 + ng) * B
                nc.vector.tensor_add(logit_sb[:, w0:w1], ps[:],
                                     rep_sb[:, w0:w1])
                eng = {"sp": nc.sync, "gp": nc.gpsimd,
                       "act": nc.scalar}[OUT_ENG[gi]]
                eng.dma_start(out=out_d[:, w0:w1],
                              in_=logit_sb[:, w0:w1])

    nc.compile()
    return nc


def _get_bass():
    if "nc" not in _CACHE:
        _CACHE["nc"] = _build_bass()
    return _CACHE["nc"]


def _q8(x, sc):
    return np.clip(np.float32(x) * sc, -15.5, 15.5).astype(E3NP)


def _interleave(a):
    """[DC*128, N] -> [128, DC*N] with [p, k*N+c] = a[k*128+p, c]."""
    n = a.shape[1]
    return np.ascontiguousarray(
        a.reshape(DC, 128, n).transpose(1, 0, 2).reshape(128, DC * n))


def _prep_inputs(inputs):
    ids = np.asarray(inputs["input_ids"])[:, 0].astype(np.int64)
    emb = np.asarray(inputs["emb"], dtype=np.float32)
    hidden = np.asarray(inputs["hidden"], dtype=np.float32)
    Wb = np.asarray(inputs["Wb"], dtype=np.float32)
    bb = np.asarray(inputs["bb"], dtype=np.float32)
    W_ih = np.asarray(inputs["W_ih"], dtype=np.float32)
    b_ih = np.asarray(inputs["b_ih"], dtype=np.float32)
    W_hh = np.asarray(inputs["W_hh"], dtype=np.float32)
    b_hh = np.asarray(inputs["b_hh"], dtype=np.float32)
    Wp = np.asarray(inputs["Wp"], dtype=np.float32)
    bp = np.asarray(inputs["bp"], dtype=np.float32)

    # hidden rows (b*64+l, d) -> [128, NT*D] with [p, t*D+d] = row t*128+p
    hid8 = _q8(np.ascontiguousarray(
        hidden.reshape(NT, 128, D).transpose(1, 0, 2).reshape(128, NT * D)),
        HSC)

    # gate weights [3D, D] (row blocks r, z, n) -> wg cols
    # [(gate g, mat m, k) -> W_m^T[k*128:(k+1)*128, g-block]]
    wg = np.empty((128, 2 * DC * G), E3NP)
    for g_ in range(3):
        for m, W in ((0, W_ih), (1, W_hh)):
            wt = _q8(W[g_ * 512:(g_ + 1) * 512, :].T, WSC)  # [D, 512]
            for k in range(DC):
                c0 = ((g_ * 2 + m) * DC + k) * 512
                wg[:, c0:c0 + 512] = wt[k * 128:(k + 1) * 128, :]

    # small-tensor block (fp16)
    smk = np.zeros((128, SM_N), np.float16)
    # x0T: [D, B] -> [128, DC*B] (relu happens on device)
    smk[:, SM_X0:SM_X0 + DC * B] = _interleave(
        np.ascontiguousarray(emb[ids].T)).astype(np.float16)
    smk[0:64, SM_WSM] = Wb[0]
    smk[64:128, SM_WSM + 1] = Wb[0]
    smk[:, SM_BB] = bb[0]
    smk[0, SM_BC:SM_BC + 2 * G] = (np.concatenate([b_ih, b_hh])
                                   * WSC).astype(np.float16)

    WpT8 = np.zeros((D, VPAD), E3NP)
    WpT8[:, :V] = _q8(Wp.T, WSC)
    bp_pad = np.zeros((VPAD,), np.float32)
    bp_pad[:V] = bp * WSC

    shared = {"hid": hid8, "wg": wg}
    in_maps = []
    for c in range(NCORES):
        sl = slice(c * VS, (c + 1) * VS)
        m = dict(shared)
        # wp cols [blk*512 + k*128 + vl] = WpT[k*128+p, blk*128+vl]
        shard = np.ascontiguousarray(WpT8[:, sl])
        m["wp"] = np.ascontiguousarray(
            shard.reshape(DC, 128, NBLK, 128)
            .transpose(1, 2, 0, 3).reshape(128, DC * VS))
        mk = smk.copy()
        # bp block tile: [p, j] = bp[c*VS + j*128 + p] * WSC
        mk[:, SM_BP:SM_BP + NBLK] = \
            bp_pad[sl].reshape(NBLK, 128).T.astype(np.float16)
        m["smk"] = mk
        in_maps.append(m)
    return in_maps


def _run(in_maps, trace=False, tmpdir=None):
    nc = _get_bass()
    return run_bass_kernel_spmd(nc, in_maps, list(range(NCORES)),
                                trace=trace, tmpdir=tmpdir)


def kernel(**inputs) -> np.ndarray:
    in_maps = _prep_inputs(inputs)
    try:
        res = _run(in_maps).results
    except Exception:
        # transient NRT device wedges have been observed on this fabric;
        # one retry after a short pause usually lands on healthy cores
        import time as _time
        _time.sleep(5.0)
        res = _run(in_maps).results
    # out[p, blk*B + b] = WSC*logits[b, c*VS + blk*128 + p]
    parts = []
    for c in range(NCORES):
        r = np.asarray(res[c]["logits"])
        parts.append(r.reshape(128, NBLK, B).transpose(2, 1, 0)
                     .reshape(B, VS))
    logits = np.concatenate(parts, axis=1)[:, :V].astype(np.float32)
    logits *= (1.0 / WSC)  # exact: power-of-two exponent shift
    return np.broadcast_to(logits[:, None, :], (B, L - 1, V))


# revision 24
# speedup vs baseline: 1.6835x; 1.0021x over previous
"""GRU-decoder first-step kernel for 8 Trainium2 NeuronCores.

Math (see reference): all L-1 output steps are identical, so compute the
single step and broadcast on host:
    x0 = relu(emb[input_ids[:, 0]])                [B, D]
    h0 = einsum("bld,l->bd", hidden, Wb[0]) + bb   [B, D]
    GRU cell (r, z, n) -> h1                       [B, D]
    logits = h1 @ Wp.T + bp                        [B, V]
    out = broadcast(logits)                        [B, L-1, V]

Sharding: tensor-parallel over the vocab dim across the 8 cores; the
(tiny) GRU math is replicated on every core.  This regime is HBM-bound,
so every large operand ships at the smallest dtype the 2e-2 error gate
allows: fp8 e3m4 (4 mantissa bits) for Wp / W_ih / W_hh / hidden with
power-of-two scales folded into the activations, fp16 for everything
error-critical (Wb, x0, biases, the GRU state) and for the logits
output (the device writes 64*logits; the host unscale is an exact
exponent shift).  ~6.4 MB per core vs 12.6 MB for the all-fp16 version.

Structure decisions, in impact order:
 - fp8 e3m4 halves the big-operand traffic; e3m4 (not e4m3) because its
   extra mantissa bit keeps the end-to-end error at ~1.5e-2;
 - the projection runs with Wp as the *stationary* operand per
   128-vocab block (out = [vocab_part, batch]): each matmul streams
   only B=32 moving rows, logits land in SBUF-native layout and are
   written as one contiguous row-DMA per flush group (host unshuffles);
 - every small tensor rides inside ONE fp16 side tensor (two early
   SWDGE loads) so no critical operand queues behind the big stream;
 - the DMA stream order is consumption order: hidden (bridge), gate
   weights r+z then n, projection blocks in shrinking groups;
 - ACT runs exactly one table set (a dummy sigmoid triggers the
   sigmoid_and_others load at t~0); all copies/elementwise run on
   DVE/Pool so no 1.3us table reload lands mid-chain;
 - projection bias is pre-replicated into a [128, 50*B] SBUF tile (one
   broadcast DVE copy) and added during the PSUM->SBUF drain, so no
   K=1 bias matmuls on the PE;
 - out-DMAs issue from the otherwise-idle sync queue: a DMA's wait
   occupies its issuing sequencer, which must not carry later work.
"""

import numpy as np
import ml_dtypes

import concourse.mybir as mybir
import concourse.tile as tile
from concourse import bacc
from concourse.bass_utils import run_bass_kernel_spmd

B, L, D, V = 32, 64, 512, 50257
NCORES = 8
VS = 6283            # per-core vocab shard; 8*6283 = 50264 >= V
NBLK = 50            # 49 full 128-col blocks + one 11-col tail block
LB = VS - 49 * 128   # last-block width (11)
VPAD = VS * NCORES
WPC = 49 * 512 + 4 * LB  # wp tensor columns per core (25132)
DC = D // 128        # 4 contraction chunks of 128
NT = (B * L) // 128  # 16 hidden row-tiles of 128
G = 3 * D
F16 = mybir.dt.float16
F32 = mybir.dt.float32
F8 = mybir.dt.float8e3          # e3m4: 4 mantissa bits, max 15.5
E3NP = ml_dtypes.float8_e3m4
WSC = 64.0           # weight scale (Wp, W_ih, W_hh, biases)
HSC = 2.0            # hidden scale

# projection DMA groups (in blocks): shrink toward the end so the
# last-arriving piece carries minimal compute + copy + out-DMA
DGRP = [16, 14, 11, 7, 2]          # wp DMA pieces (blocks)
# compute groups are OFFSET from the DMA pieces: each group's FIRST
# block sits in an already-arrived piece, so the scheduler's hoisted
# group-head Ldweights never head-of-line blocks the PE wait queue
# (which would defer the previous group's drain past the next DMA)
CGRP = [14, 14, 11, 7, 3, 1]
CGO = [sum(CGRP[:i]) for i in range(len(CGRP) + 1)]
# out-DMA issue queue per group: early groups ride SWDGE (whose
# transfers yield to the HWDGE input stream in arbitration), the tail
# two ride SP, the sole HWDGE user at that point
OUT_ENG = ["gp", "gp", "gp", "gp", "sp", "sp"]


# smk column map: x0T | wsm | bb | bpT | bcat-row
SM_X0, SM_WSM, SM_BB, SM_BP = 0, DC * B, DC * B + 2, DC * B + 3
SM_BC = SM_BP + NBLK          # 181
SM_N = SM_BC + 2 * G          # 3253

_CACHE: dict = {}


def _build_bass():
    nc = bacc.Bacc("TRN2", target_bir_lowering=False, debug=False,
                   num_devices=NCORES)

    hid_d = nc.dram_tensor("hid", [128, NT * D], F8, kind="ExternalInput")
    # gate weights, consumption order r, z, n; cols
    # [(gate, mat, k) -> 512 j-cols]: block index = (go*2 + m)*DC + k
    wg_d = nc.dram_tensor("wg", [128, 2 * DC * G], F8, kind="ExternalInput")
    wp_d = nc.dram_tensor("wp", [128, WPC], F8, kind="ExternalInput")
    smk_d = nc.dram_tensor("smk", [128, SM_N], F16, kind="ExternalInput")
    # out[p, blk*B + b] = WSC * logits[b, blk*128 + p]
    out_d = nc.dram_tensor("logits", [128, NBLK * B], F16,
                           kind="ExternalOutput")

    AF = mybir.ActivationFunctionType
    ALU = mybir.AluOpType
    GW = DC * 512  # cols per (gate, mat)

    with tile.TileContext(nc) as tc:
        with (
            tc.tile_pool(name="wp", bufs=1) as wp_pool,
            tc.tile_pool(name="big", bufs=1) as big,
            tc.tile_pool(name="sm", bufs=1) as sm,
            tc.tile_pool(name="ps_mm", bufs=4, space="PSUM") as ps_mm,
            tc.tile_pool(name="ps_g", bufs=2, space="PSUM") as ps_g,
            tc.tile_pool(name="ps_b", bufs=1, space="PSUM") as ps_b,
        ):
            # ---- x0/wsm/bb/bp ride an early SWDGE queue (lands in the
            # first inter-transfer gap of the big stream) ----
            smk_sb = big.tile([128, SM_N], F16, tag="smk")
            nc.gpsimd.dma_start(out=smk_sb[:, 0:SM_BC],
                                in_=smk_d[:, 0:SM_BC])

            # ---- big stream on the sync queue, consumption order;
            # the bcat bias row goes 3rd on SP (tiny transfer, FIFO
            # grant right behind hid) because a hoisted gate-bias
            # Ldweights head-of-line blocks the PE wait queue until
            # bcat lands ----
            hid_sb = big.tile([128, NT * D], F8, tag="hid")
            HHALF = (NT // 2) * D
            nc.sync.dma_start(out=hid_sb[:, 0:HHALF], in_=hid_d[:, 0:HHALF])
            nc.sync.dma_start(out=hid_sb[:, HHALF:], in_=hid_d[:, HHALF:])
            nc.sync.dma_start(out=smk_sb[0:1, SM_BC:],
                              in_=smk_d[0:1, SM_BC:])
            wg_sb = big.tile([128, 2 * DC * G], F8, tag="wg")
            # r+z gates first (2 gates * 2 mats * GW cols), n last
            nc.sync.dma_start(out=wg_sb[:, 0:4 * GW], in_=wg_d[:, 0:4 * GW])
            nc.sync.dma_start(out=wg_sb[:, 4 * GW:], in_=wg_d[:, 4 * GW:])
            wp_sb = wp_pool.tile([128, WPC], F8, tag="wp")
            dgo = 0
            for ng in DGRP:
                lo = dgo * 512
                hi = min((dgo + ng) * 512, WPC)
                nc.sync.dma_start(out=wp_sb[:, lo:hi], in_=wp_d[:, lo:hi])
                dgo += ng

            ones_sb = sm.tile([1, B], F16, tag="ones")
            nc.vector.memset(ones_sb[:], 1.0)
            # dummy sigmoid: pulls the ACT sigmoid_and_others table load
            # (1.28us) to t~0; that set covers relu/identity/tanh/sigmoid
            # so no reload ever lands on the critical chain
            dum_sb = sm.tile([1, 1], F32, tag="dum")
            nc.scalar.activation(dum_sb[:], ones_sb[0:1, 0:1], AF.Sigmoid)

            # projection bias, replicated [128, NBLK*B] with one
            # broadcast copy (added during the PSUM->SBUF drain)
            rep_sb = big.tile([128, NBLK * B], F16, tag="rep")
            nc.vector.tensor_copy(
                rep_sb[:, :].rearrange("p (j b) -> p j b", b=B),
                smk_sb[:, SM_BP:SM_BP + NBLK].unsqueeze(2)
                .broadcast_to([128, NBLK, B]))

            # ==== GRU cell in transposed space: [d, b] tiles are
            # [128, DC*B] with column = k*B + b. ====

            # x0T = relu(embT rows) -> fp16
            x0t_sb = sm.tile([128, DC * B], F16, tag="x0t")
            nc.scalar.activation(x0t_sb[:], smk_sb[:, SM_X0:SM_X0 + DC * B],
                                 AF.Relu)

            # bridge: h0T[d, 2t+j] = sum_p hid_tile[t][p, d] * wsm[p, j]
            # (rows t*128+p of hidden are (b=2t+p//64, l=p%64))
            ps_h0 = ps_b.tile([128, DC * B], F32, tag="b")
            wsm = smk_sb[:, SM_WSM:SM_WSM + 2]
            for k in range(DC):
                for t_i in range(NT):
                    nc.tensor.matmul(
                        ps_h0[:, k * B + 2 * t_i:k * B + 2 * t_i + 2],
                        hid_sb[:, t_i * D + k * 128:t_i * D + (k + 1) * 128],
                        wsm,
                        start=True, stop=True,
                    )
            # h0 = ps/HSC + bb (f32 master + fp16 feed for the PE)
            h0t_sb = sm.tile([128, DC * B], F32, tag="h0t")
            nc.scalar.activation(h0t_sb[:], ps_h0[:], AF.Identity,
                                 bias=smk_sb[:, SM_BB:SM_BB + 1],
                                 scale=1.0 / HSC)
            h0t16 = sm.tile([128, DC * B], F16, tag="h0t16")
            nc.vector.tensor_copy(h0t16[:], h0t_sb[:])

            # gates: gate^T[j, b] per (jb, k); lhsT = W^T [d, j-block]
            # (fp8, scaled by WSC), rhs = x0T / h0T (fp16). Biases
            # (host-scaled by WSC) ride K=1 matmuls with values along M.
            def gate_psum(g_, use_x, use_h):
                # g_ is the position in wg AND the pytorch row block
                # (both are r, z, n order)
                ps = ps_g.tile([128, DC * B], F32, tag="g")
                for jb in range(DC):
                    o = ps[:, jb * B:(jb + 1) * B]
                    bw = SM_BC + g_ * 512 + jb * 128
                    ops = []
                    if use_x:
                        ops.append((smk_sb[0:1, bw:bw + 128], ones_sb[:]))
                    if use_h:
                        ops.append((smk_sb[0:1, G + bw:G + bw + 128],
                                    ones_sb[:]))
                    for k in range(DC):
                        if use_x:
                            cw = (g_ * 2 * DC + k) * 512 + jb * 128
                            ops.append((wg_sb[:, cw:cw + 128],
                                        x0t_sb[:, k * B:(k + 1) * B]))
                        if use_h:
                            cw = ((g_ * 2 + 1) * DC + k) * 512 + jb * 128
                            ops.append((wg_sb[:, cw:cw + 128],
                                        h0t16[:, k * B:(k + 1) * B]))
                    for i, (lhsT, rhs) in enumerate(ops):
                        nc.tensor.matmul(o, lhsT, rhs, start=(i == 0),
                                         stop=(i == len(ops) - 1))
                return ps

            # r and z first (their weights arrive first); z's products
            # with h0 precompute while the n-gate weights land
            ps_r = gate_psum(0, True, True)
            r_sb = sm.tile([128, DC * B], F32, tag="r")
            nc.scalar.activation(r_sb[:], ps_r[:], AF.Sigmoid,
                                 scale=1.0 / WSC)
            ps_z = gate_psum(1, True, True)
            z_sb = sm.tile([128, DC * B], F32, tag="z")
            nc.scalar.activation(z_sb[:], ps_z[:], AF.Sigmoid,
                                 scale=1.0 / WSC)
            # zh0 = z * h0, zc = 1 - z (both before n is ready)
            zh0_sb = sm.tile([128, DC * B], F32, tag="zh0")
            nc.vector.tensor_mul(zh0_sb[:], z_sb[:], h0t_sb[:])
            zc_sb = sm.tile([128, DC * B], F32, tag="zc")
            nc.vector.tensor_scalar(zc_sb[:], z_sb[:], -1.0, 1.0,
                                    ALU.mult, ALU.add)

            ps_xn = gate_psum(2, True, False)
            ps_hn = gate_psum(2, False, True)
            t1 = sm.tile([128, DC * B], F32, tag="t1")
            nc.vector.tensor_mul(t1[:], r_sb[:], ps_hn[:])
            t2 = sm.tile([128, DC * B], F32, tag="t2")
            nc.vector.tensor_add(t2[:], t1[:], ps_xn[:])
            n_sb = sm.tile([128, DC * B], F32, tag="n")
            nc.scalar.activation(n_sb[:], t2[:], AF.Tanh, scale=1.0 / WSC)

            # h1 = (1-z)*n + z*h0, fused straight into the fp16 PE feed
            f_sb = sm.tile([128, DC * B], F32, tag="f")
            nc.vector.tensor_mul(f_sb[:], zc_sb[:], n_sb[:])
            h1t_sb = sm.tile([128, DC * B], F16, tag="h1t")
            nc.vector.tensor_add(h1t_sb[:], f_sb[:], zh0_sb[:])

            # ---- projection: per 128-vocab block, Wp is stationary
            # (lhsT [128 d, 128 v]) and h1T moves (rhs [128 d, B]);
            # ps[v, b] accumulates the 4 k-chunks; the PSUM->SBUF drain
            # adds the replicated bias. Out-DMAs ride the idle sync
            # queue so their waits never block a compute engine. ----
            logit_sb = big.tile([128, NBLK * B], F16, tag="lg")
            for gi, ng in enumerate(CGRP):
                b0 = CGO[gi]
                ps = ps_mm.tile([128, ng * B], F32, tag="mm")
                npart = 128
                for i in range(ng):
                    blk = b0 + i
                    bw = 128 if blk < 49 else LB   # tail block is narrow
                    npart = bw
                    o = ps[0:bw, i * B:(i + 1) * B]
                    for k in range(DC):
                        nc.tensor.matmul(
                            o,
                            wp_sb[:, blk * 512 + k * bw:
                                  blk * 512 + (k + 1) * bw],
                            h1t_sb[:, k * B:(k + 1) * B],
                            start=(k == 0), stop=(k == DC - 1))
                # drains on DVE (fastest copy engine); out-DMAs spread
                # per OUT_ENG so no single issue path serializes the tail
                w0, w1 = b0 * B, (b0 + ng) * B
                np_ = npart if ng == 1 else 128
                nc.vector.tensor_add(logit_sb[0:np_, w0:w1],
                                     ps[0:np_, :],
                                     rep_sb[0:np_, w0:w1])
                eng = {"sp": nc.sync, "gp": nc.gpsimd,
                       "act": nc.scalar}[OUT_ENG[gi]]
                eng.dma_start(out=out_d[0:np_, w0:w1],
                              in_=logit_sb[0:np_, w0:w1])

    nc.compile()
    return nc


def _get_bass():
    if "nc" not in _CACHE:
        _CACHE["nc"] = _build_bass()
    return _CACHE["nc"]


def _q8(x, sc):
    return np.clip(np.float32(x) * sc, -15.5, 15.5).astype(E3NP)


def _interleave(a):
    """[DC*128, N] -> [128, DC*N] with [p, k*N+c] = a[k*128+p, c]."""
    n = a.shape[1]
    return np.ascontiguousarray(
        a.reshape(DC, 128, n).transpose(1, 0, 2).reshape(128, DC * n))


def _prep_inputs(inputs):
    ids = np.asarray(inputs["input_ids"])[:, 0].astype(np.int64)
    emb = np.asarray(inputs["emb"], dtype=np.float32)
    hidden = np.asarray(inputs["hidden"], dtype=np.float32)
    Wb = np.asarray(inputs["Wb"], dtype=np.float32)
    bb = np.asarray(inputs["bb"], dtype=np.float32)
    W_ih = np.asarray(inputs["W_ih"], dtype=np.float32)
    b_ih = np.asarray(inputs["b_ih"], dtype=np.float32)
    W_hh = np.asarray(inputs["W_hh"], dtype=np.float32)
    b_hh = np.asarray(inputs["b_hh"], dtype=np.float32)
    Wp = np.asarray(inputs["Wp"], dtype=np.float32)
    bp = np.asarray(inputs["bp"], dtype=np.float32)

    # hidden rows (b*64+l, d) -> [128, NT*D] with [p, t*D+d] = row t*128+p
    hid8 = _q8(np.ascontiguousarray(
        hidden.reshape(NT, 128, D).transpose(1, 0, 2).reshape(128, NT * D)),
        HSC)

    # gate weights [3D, D] (row blocks r, z, n) -> wg cols
    # [(gate g, mat m, k) -> W_m^T[k*128:(k+1)*128, g-block]]
    wg = np.empty((128, 2 * DC * G), E3NP)
    for g_ in range(3):
        for m, W in ((0, W_ih), (1, W_hh)):
            wt = _q8(W[g_ * 512:(g_ + 1) * 512, :].T, WSC)  # [D, 512]
            for k in range(DC):
                c0 = ((g_ * 2 + m) * DC + k) * 512
                wg[:, c0:c0 + 512] = wt[k * 128:(k + 1) * 128, :]

    # small-tensor block (fp16)
    smk = np.zeros((128, SM_N), np.float16)
    # x0T: [D, B] -> [128, DC*B] (relu happens on device)
    smk[:, SM_X0:SM_X0 + DC * B] = _interleave(
        np.ascontiguousarray(emb[ids].T)).astype(np.float16)
    smk[0:64, SM_WSM] = Wb[0]
    smk[64:128, SM_WSM + 1] = Wb[0]
    smk[:, SM_BB] = bb[0]
    smk[0, SM_BC:SM_BC + 2 * G] = (np.concatenate([b_ih, b_hh])
                                   * WSC).astype(np.float16)

    WpT8 = np.zeros((D, VPAD), E3NP)
    WpT8[:, :V] = _q8(Wp.T, WSC)
    bp_pad = np.zeros((NCORES * NBLK * 128,), np.float32)
    for c in range(NCORES):
        bp_c = np.zeros((NBLK * 128,), np.float32)
        seg = bp[c * VS:(c + 1) * VS] if c * VS < V else \
            np.zeros((0,), np.float32)
        seg = np.pad(seg * WSC, (0, VS - seg.shape[0]))
        bp_c[:49 * 128] = seg[:49 * 128]
        bp_c[49 * 128:49 * 128 + LB] = seg[49 * 128:]
        bp_pad[c * NBLK * 128:(c + 1) * NBLK * 128] = bp_c

    shared = {"hid": hid8, "wg": wg}
    in_maps = []
    for c in range(NCORES):
        sl = slice(c * VS, (c + 1) * VS)
        m = dict(shared)
        # wp cols [blk*512 + k*bw + vl] = WpT[k*128+p, blk*128+vl]
        shard = np.ascontiguousarray(WpT8[:, sl])
        wpk = np.empty((128, WPC), E3NP)
        wpk[:, :49 * 512] = np.ascontiguousarray(
            shard[:, :49 * 128].reshape(DC, 128, 49, 128)
            .transpose(1, 2, 0, 3).reshape(128, 49 * 512))
        wpk[:, 49 * 512:] = np.ascontiguousarray(
            shard[:, 49 * 128:].reshape(DC, 128, LB)
            .transpose(1, 0, 2).reshape(128, DC * LB))
        m["wp"] = wpk
        mk = smk.copy()
        # bp block tile: [p, j] = bp[block j, row p] * WSC
        mk[:, SM_BP:SM_BP + NBLK] = \
            bp_pad[c * NBLK * 128:(c + 1) * NBLK * 128] \
            .reshape(NBLK, 128).T.astype(np.float16)
        m["smk"] = mk
        in_maps.append(m)
    return in_maps


def _run(in_maps, trace=False, tmpdir=None):
    nc = _get_bass()
    return run_bass_kernel_spmd(nc, in_maps, list(range(NCORES)),
                                trace=trace, tmpdir=tmpdir)


def kernel(**inputs) -> np.ndarray:
    in_maps = _prep_inputs(inputs)
    try:
        res = _run(in_maps).results
    except Exception:
        # transient NRT device wedges have been observed on this fabric;
        # one retry after a short pause usually lands on healthy cores
        import time as _time
        _time.sleep(5.0)
        res = _run(in_maps).results
    # out[p, blk*B + b] = WSC*logits[b, c*VS + blk*128 + p]
    parts = []
    for c in range(NCORES):
        r = np.asarray(res[c]["logits"])
        full = (r[:, :49 * B].reshape(128, 49, B).transpose(2, 1, 0)
                .reshape(B, 49 * 128))
        tail = r[0:LB, 49 * B:50 * B].T            # [B, LB]
        parts.append(np.concatenate([full, tail], axis=1))
    logits = np.concatenate(parts, axis=1)[:, :V].astype(np.float32)
    logits *= (1.0 / WSC)  # exact: power-of-two exponent shift
    return np.broadcast_to(logits[:, None, :], (B, L - 1, V))


# revision 32
# speedup vs baseline: 1.6915x; 1.0047x over previous
"""GRU-decoder first-step kernel for 8 Trainium2 NeuronCores.

Math (see reference): all L-1 output steps are identical, so compute the
single step and broadcast on host:
    x0 = relu(emb[input_ids[:, 0]])                [B, D]
    h0 = einsum("bld,l->bd", hidden, Wb[0]) + bb   [B, D]
    GRU cell (r, z, n) -> h1                       [B, D]
    logits = h1 @ Wp.T + bp                        [B, V]
    out = broadcast(logits)                        [B, L-1, V]

Sharding: tensor-parallel over the vocab dim across the 8 cores; the
(tiny) GRU math is replicated on every core.  This regime is HBM-bound,
so every large operand ships at the smallest dtype the 2e-2 error gate
allows: fp8 e3m4 (4 mantissa bits) for Wp / W_ih / W_hh / hidden with
power-of-two scales folded into the activations, fp16 for everything
error-critical (Wb, x0, biases, the GRU state) and for the logits
output (the device writes 64*logits; the host unscale is an exact
exponent shift).  ~6.4 MB per core vs 12.6 MB for the all-fp16 version.

Structure decisions, in impact order (TimelineSim: 40.5us baseline ->
24.0us; ~18.2us of that is the irreducible 6.3 MB DMA stream):
 - fp8 e3m4 halves the big-operand traffic; e3m4 (not e4m3) because its
   extra mantissa bit keeps the end-to-end error at ~1.5e-2;
 - the projection runs with Wp as the *stationary* operand per
   128-vocab block (out = [vocab_part, batch]): each matmul streams
   only B=32 moving rows (engine cost is per moving row), logits land
   in SBUF-native layout and flush as contiguous row-DMAs per group
   (host unshuffles);
 - the DMA stream order is consumption order: hidden (bridge), the
   bcat bias row (a tiny SP transfer - a hoisted gate-bias Ldweights
   head-of-line blocks the 4-deep PE wait queue until it lands), gate
   weights r+z then n, projection pieces in shrinking groups; other
   small tensors ride one early-SWDGE fp16 tensor whose transfer slips
   into the first inter-piece gap;
 - ACT runs exactly one table set (a dummy sigmoid pulls the
   sigmoid_and_others load, which covers relu/identity/tanh too, to
   t~0); all copies/elementwise run on DVE so no 1.3us table reload
   lands mid-chain, and the GRU tail is restructured as
   h1 = (1-z)*n + z*h0 with both z-products precomputed while the
   n-gate weights land, h1 fused straight into the fp16 PE feed;
 - projection bias is pre-replicated into a [128, 50*B] SBUF tile (one
   broadcast DVE copy) and added during the PSUM->SBUF drain, so no
   K=1 bias matmuls on the PE;
 - compute groups are OFFSET half a piece from the DMA pieces and
   out-DMAs alternate SWDGE/SP queues: a DMA's semaphore wait occupies
   its issuing sequencer, SWDGE transfers yield to the input stream in
   DMA-engine arbitration, and each drain's PE-count wait covers
   everything the scheduler placed before it (~one piece of lag);
 - the vocab shard is unpadded (6283 = 49 full blocks + one 11-wide
   tail block) so the last-arriving piece is minimal.
"""

import numpy as np
import ml_dtypes

import concourse.mybir as mybir
import concourse.tile as tile
from concourse import bacc
from concourse.bass_utils import run_bass_kernel_spmd

B, L, D, V = 32, 64, 512, 50257
NCORES = 8
VS = 6283            # per-core vocab shard; 8*6283 = 50264 >= V
NBLK = 50            # 49 full 128-col blocks + one 11-col tail block
LB = VS - 49 * 128   # last-block width (11)
VPAD = VS * NCORES
WPC = 49 * 512 + 4 * LB  # wp tensor columns per core (25132)
DC = D // 128        # 4 contraction chunks of 128
NT = (B * L) // 128  # 16 hidden row-tiles of 128
G = 3 * D
F16 = mybir.dt.float16
F32 = mybir.dt.float32
F8 = mybir.dt.float8e3          # e3m4: 4 mantissa bits, max 15.5
E3NP = ml_dtypes.float8_e3m4
WSC = 64.0           # weight scale (Wp, W_ih, W_hh, biases)
HSC = 2.0            # hidden scale

# projection DMA groups (in blocks): shrink toward the end so the
# last-arriving piece carries minimal compute + copy + out-DMA
DGRP = [16, 14, 11, 7, 2]          # wp DMA pieces (blocks)
# compute groups are OFFSET from the DMA pieces: each group's FIRST
# block sits in an already-arrived piece, so the scheduler's hoisted
# group-head Ldweights never head-of-line blocks the PE wait queue
# (which would defer the previous group's drain past the next DMA)
CGRP = [14, 14, 11, 7, 3, 1]
CGO = [sum(CGRP[:i]) for i in range(len(CGRP) + 1)]
# out-DMA issue queue per group: early groups ride SWDGE (whose
# transfers yield to the HWDGE input stream in arbitration), the tail
# splits across SP and Pool so no single issue path serializes
OUT_ENG = ["gp", "sp", "gp", "sp", "gp", "sp"]


# smk column map: x0T | wsm | bb | bpT | bcat-row
SM_X0, SM_WSM, SM_BB, SM_BP = 0, DC * B, DC * B + 2, DC * B + 3
SM_BC = SM_BP + NBLK          # 181
SM_N = SM_BC + 2 * G          # 3253

_CACHE: dict = {}


def _build_bass():
    nc = bacc.Bacc("TRN2", target_bir_lowering=False, debug=False,
                   num_devices=NCORES)

    hid_d = nc.dram_tensor("hid", [128, NT * D], F8, kind="ExternalInput")
    # gate weights, consumption order r, z, n; cols
    # [(gate, mat, k) -> 512 j-cols]: block index = (go*2 + m)*DC + k
    wg_d = nc.dram_tensor("wg", [128, 2 * DC * G], F8, kind="ExternalInput")
    wp_d = nc.dram_tensor("wp", [128, WPC], F8, kind="ExternalInput")
    smk_d = nc.dram_tensor("smk", [128, SM_N], F16, kind="ExternalInput")
    # out[p, blk*B + b] = WSC * logits[b, blk*128 + p]
    out_d = nc.dram_tensor("logits", [128, NBLK * B], F16,
                           kind="ExternalOutput")

    AF = mybir.ActivationFunctionType
    ALU = mybir.AluOpType
    GW = DC * 512  # cols per (gate, mat)

    with tile.TileContext(nc) as tc:
        with (
            tc.tile_pool(name="wp", bufs=1) as wp_pool,
            tc.tile_pool(name="big", bufs=1) as big,
            tc.tile_pool(name="sm", bufs=1) as sm,
            tc.tile_pool(name="ps_mm", bufs=4, space="PSUM") as ps_mm,
            tc.tile_pool(name="ps_g", bufs=2, space="PSUM") as ps_g,
            tc.tile_pool(name="ps_b", bufs=1, space="PSUM") as ps_b,
        ):
            # ---- x0/wsm/bb/bp ride an early SWDGE queue (lands in the
            # first inter-transfer gap of the big stream) ----
            smk_sb = big.tile([128, SM_N], F16, tag="smk")
            nc.gpsimd.dma_start(out=smk_sb[:, 0:SM_BC],
                                in_=smk_d[:, 0:SM_BC])

            # ---- big stream on the sync queue, consumption order;
            # the bcat bias row goes 3rd on SP (tiny transfer, FIFO
            # grant right behind hid) because a hoisted gate-bias
            # Ldweights head-of-line blocks the PE wait queue until
            # bcat lands ----
            hid_sb = big.tile([128, NT * D], F8, tag="hid")
            HHALF = (NT // 2) * D
            nc.sync.dma_start(out=hid_sb[:, 0:HHALF], in_=hid_d[:, 0:HHALF])
            nc.sync.dma_start(out=hid_sb[:, HHALF:], in_=hid_d[:, HHALF:])
            nc.sync.dma_start(out=smk_sb[0:1, SM_BC:],
                              in_=smk_d[0:1, SM_BC:])
            wg_sb = big.tile([128, 2 * DC * G], F8, tag="wg")
            # r+z gates first (2 gates * 2 mats * GW cols), n last
            nc.sync.dma_start(out=wg_sb[:, 0:4 * GW], in_=wg_d[:, 0:4 * GW])
            nc.sync.dma_start(out=wg_sb[:, 4 * GW:], in_=wg_d[:, 4 * GW:])
            wp_sb = wp_pool.tile([128, WPC], F8, tag="wp")
            dgo = 0
            for ng in DGRP:
                lo = dgo * 512
                hi = min((dgo + ng) * 512, WPC)
                nc.sync.dma_start(out=wp_sb[:, lo:hi], in_=wp_d[:, lo:hi])
                dgo += ng

            ones_sb = sm.tile([1, B], F16, tag="ones")
            nc.vector.memset(ones_sb[:], 1.0)
            # dummy sigmoid: pulls the ACT sigmoid_and_others table load
            # (1.28us) to t~0; that set covers relu/identity/tanh/sigmoid
            # so no reload ever lands on the critical chain
            dum_sb = sm.tile([1, 1], F32, tag="dum")
            nc.scalar.activation(dum_sb[:], ones_sb[0:1, 0:1], AF.Sigmoid)

            # projection bias, replicated [128, NBLK*B] with one
            # broadcast copy (added during the PSUM->SBUF drain)
            rep_sb = big.tile([128, NBLK * B], F16, tag="rep")
            nc.vector.tensor_copy(
                rep_sb[:, :].rearrange("p (j b) -> p j b", b=B),
                smk_sb[:, SM_BP:SM_BP + NBLK].unsqueeze(2)
                .broadcast_to([128, NBLK, B]))

            # ==== GRU cell in transposed space: [d, b] tiles are
            # [128, DC*B] with column = k*B + b. ====

            # x0T = relu(embT rows) -> fp16
            x0t_sb = sm.tile([128, DC * B], F16, tag="x0t")
            nc.scalar.activation(x0t_sb[:], smk_sb[:, SM_X0:SM_X0 + DC * B],
                                 AF.Relu)

            # bridge: h0T[d, 2t+j] = sum_p hid_tile[t][p, d] * wsm[p, j]
            # (rows t*128+p of hidden are (b=2t+p//64, l=p%64))
            ps_h0 = ps_b.tile([128, DC * B], F32, tag="b")
            wsm = smk_sb[:, SM_WSM:SM_WSM + 2]
            for k in range(DC):
                for t_i in range(NT):
                    nc.tensor.matmul(
                        ps_h0[:, k * B + 2 * t_i:k * B + 2 * t_i + 2],
                        hid_sb[:, t_i * D + k * 128:t_i * D + (k + 1) * 128],
                        wsm,
                        start=True, stop=True,
                    )
            # h0 = ps/HSC + bb (f32 master + fp16 feed for the PE)
            h0t_sb = sm.tile([128, DC * B], F32, tag="h0t")
            nc.scalar.activation(h0t_sb[:], ps_h0[:], AF.Identity,
                                 bias=smk_sb[:, SM_BB:SM_BB + 1],
                                 scale=1.0 / HSC)
            h0t16 = sm.tile([128, DC * B], F16, tag="h0t16")
            nc.vector.tensor_copy(h0t16[:], h0t_sb[:])

            # gates: gate^T[j, b] per (jb, k); lhsT = W^T [d, j-block]
            # (fp8, scaled by WSC), rhs = x0T / h0T (fp16). Biases
            # (host-scaled by WSC) ride K=1 matmuls with values along M.
            def gate_psum(g_, use_x, use_h):
                # g_ is the position in wg AND the pytorch row block
                # (both are r, z, n order)
                ps = ps_g.tile([128, DC * B], F32, tag="g")
                for jb in range(DC):
                    o = ps[:, jb * B:(jb + 1) * B]
                    bw = SM_BC + g_ * 512 + jb * 128
                    ops = []
                    if use_x:
                        ops.append((smk_sb[0:1, bw:bw + 128], ones_sb[:]))
                    if use_h:
                        ops.append((smk_sb[0:1, G + bw:G + bw + 128],
                                    ones_sb[:]))
                    for k in range(DC):
                        if use_x:
                            cw = (g_ * 2 * DC + k) * 512 + jb * 128
                            ops.append((wg_sb[:, cw:cw + 128],
                                        x0t_sb[:, k * B:(k + 1) * B]))
                        if use_h:
                            cw = ((g_ * 2 + 1) * DC + k) * 512 + jb * 128
                            ops.append((wg_sb[:, cw:cw + 128],
                                        h0t16[:, k * B:(k + 1) * B]))
                    for i, (lhsT, rhs) in enumerate(ops):
                        nc.tensor.matmul(o, lhsT, rhs, start=(i == 0),
                                         stop=(i == len(ops) - 1))
                return ps

            # r and z first (their weights arrive first); z's products
            # with h0 precompute while the n-gate weights land
            ps_r = gate_psum(0, True, True)
            r_sb = sm.tile([128, DC * B], F32, tag="r")
            nc.scalar.activation(r_sb[:], ps_r[:], AF.Sigmoid,
                                 scale=1.0 / WSC)
            ps_z = gate_psum(1, True, True)
            z_sb = sm.tile([128, DC * B], F32, tag="z")
            nc.scalar.activation(z_sb[:], ps_z[:], AF.Sigmoid,
                                 scale=1.0 / WSC)
            # zh0 = z * h0, zc = 1 - z (both before n is ready)
            zh0_sb = sm.tile([128, DC * B], F32, tag="zh0")
            nc.vector.tensor_mul(zh0_sb[:], z_sb[:], h0t_sb[:])
            zc_sb = sm.tile([128, DC * B], F32, tag="zc")
            nc.vector.tensor_scalar(zc_sb[:], z_sb[:], -1.0, 1.0,
                                    ALU.mult, ALU.add)

            ps_xn = gate_psum(2, True, False)
            ps_hn = gate_psum(2, False, True)
            t1 = sm.tile([128, DC * B], F32, tag="t1")
            nc.vector.tensor_mul(t1[:], r_sb[:], ps_hn[:])
            t2 = sm.tile([128, DC * B], F32, tag="t2")
            nc.vector.tensor_add(t2[:], t1[:], ps_xn[:])
            n_sb = sm.tile([128, DC * B], F32, tag="n")
            nc.scalar.activation(n_sb[:], t2[:], AF.Tanh, scale=1.0 / WSC)

            # h1 = (1-z)*n + z*h0, fused straight into the fp16 PE feed
            f_sb = sm.tile([128, DC * B], F32, tag="f")
            nc.vector.tensor_mul(f_sb[:], zc_sb[:], n_sb[:])
            h1t_sb = sm.tile([128, DC * B], F16, tag="h1t")
            nc.vector.tensor_add(h1t_sb[:], f_sb[:], zh0_sb[:])

            # ---- projection: per 128-vocab block, Wp is stationary
            # (lhsT [128 d, 128 v]) and h1T moves (rhs [128 d, B]);
            # ps[v, b] accumulates the 4 k-chunks; the PSUM->SBUF drain
            # adds the replicated bias. Out-DMAs ride the idle sync
            # queue so their waits never block a compute engine. ----
            logit_sb = big.tile([128, NBLK * B], F16, tag="lg")
            for gi, ng in enumerate(CGRP):
                b0 = CGO[gi]
                ps = ps_mm.tile([128, ng * B], F32, tag="mm")
                npart = 128
                for i in range(ng):
                    blk = b0 + i
                    bw = 128 if blk < 49 else LB   # tail block is narrow
                    npart = bw
                    o = ps[0:bw, i * B:(i + 1) * B]
                    for k in range(DC):
                        nc.tensor.matmul(
                            o,
                            wp_sb[:, blk * 512 + k * bw:
                                  blk * 512 + (k + 1) * bw],
                            h1t_sb[:, k * B:(k + 1) * B],
                            start=(k == 0), stop=(k == DC - 1))
                # drains on DVE (fastest copy engine); out-DMAs spread
                # per OUT_ENG so no single issue path serializes the tail
                w0, w1 = b0 * B, (b0 + ng) * B
                np_ = npart if ng == 1 else 128
                nc.vector.tensor_add(logit_sb[0:np_, w0:w1],
                                     ps[0:np_, :],
                                     rep_sb[0:np_, w0:w1])
                eng = {"sp": nc.sync, "gp": nc.gpsimd,
                       "act": nc.scalar}[OUT_ENG[gi]]
                eng.dma_start(out=out_d[0:np_, w0:w1],
                              in_=logit_sb[0:np_, w0:w1])

    nc.compile()
    return nc


def _get_bass():
    if "nc" not in _CACHE:
        _CACHE["nc"] = _build_bass()
    return _CACHE["nc"]


def _q8(x, sc):
    return np.clip(np.float32(x) * sc, -15.5, 15.5).astype(E3NP)


def _interleave(a):
    """[DC*128, N] -> [128, DC*N] with [p, k*N+c] = a[k*128+p, c]."""
    n = a.shape[1]
    return np.ascontiguousarray(
        a.reshape(DC, 128, n).transpose(1, 0, 2).reshape(128, DC * n))


def _prep_inputs(inputs):
    ids = np.asarray(inputs["input_ids"])[:, 0].astype(np.int64)
    emb = np.asarray(inputs["emb"], dtype=np.float32)
    hidden = np.asarray(inputs["hidden"], dtype=np.float32)
    Wb = np.asarray(inputs["Wb"], dtype=np.float32)
    bb = np.asarray(inputs["bb"], dtype=np.float32)
    W_ih = np.asarray(inputs["W_ih"], dtype=np.float32)
    b_ih = np.asarray(inputs["b_ih"], dtype=np.float32)
    W_hh = np.asarray(inputs["W_hh"], dtype=np.float32)
    b_hh = np.asarray(inputs["b_hh"], dtype=np.float32)
    Wp = np.asarray(inputs["Wp"], dtype=np.float32)
    bp = np.asarray(inputs["bp"], dtype=np.float32)

    # hidden rows (b*64+l, d) -> [128, NT*D] with [p, t*D+d] = row t*128+p
    hid8 = _q8(np.ascontiguousarray(
        hidden.reshape(NT, 128, D).transpose(1, 0, 2).reshape(128, NT * D)),
        HSC)

    # gate weights [3D, D] (row blocks r, z, n) -> wg cols
    # [(gate g, mat m, k) -> W_m^T[k*128:(k+1)*128, g-block]]
    wg = np.empty((128, 2 * DC * G), E3NP)
    for g_ in range(3):
        for m, W in ((0, W_ih), (1, W_hh)):
            wt = _q8(W[g_ * 512:(g_ + 1) * 512, :].T, WSC)  # [D, 512]
            for k in range(DC):
                c0 = ((g_ * 2 + m) * DC + k) * 512
                wg[:, c0:c0 + 512] = wt[k * 128:(k + 1) * 128, :]

    # small-tensor block (fp16)
    smk = np.zeros((128, SM_N), np.float16)
    # x0T: [D, B] -> [128, DC*B] (relu happens on device)
    smk[:, SM_X0:SM_X0 + DC * B] = _interleave(
        np.ascontiguousarray(emb[ids].T)).astype(np.float16)
    smk[0:64, SM_WSM] = Wb[0]
    smk[64:128, SM_WSM + 1] = Wb[0]
    smk[:, SM_BB] = bb[0]
    smk[0, SM_BC:SM_BC + 2 * G] = (np.concatenate([b_ih, b_hh])
                                   * WSC).astype(np.float16)

    WpT8 = np.zeros((D, VPAD), E3NP)
    WpT8[:, :V] = _q8(Wp.T, WSC)
    bp_pad = np.zeros((NCORES * NBLK * 128,), np.float32)
    for c in range(NCORES):
        bp_c = np.zeros((NBLK * 128,), np.float32)
        seg = bp[c * VS:(c + 1) * VS] if c * VS < V else \
            np.zeros((0,), np.float32)
        seg = np.pad(seg * WSC, (0, VS - seg.shape[0]))
        bp_c[:49 * 128] = seg[:49 * 128]
        bp_c[49 * 128:49 * 128 + LB] = seg[49 * 128:]
        bp_pad[c * NBLK * 128:(c + 1) * NBLK * 128] = bp_c

    shared = {"hid": hid8, "wg": wg}
    in_maps = []
    for c in range(NCORES):
        sl = slice(c * VS, (c + 1) * VS)
        m = dict(shared)
        # wp cols [blk*512 + k*bw + vl] = WpT[k*128+p, blk*128+vl]
        shard = np.ascontiguousarray(WpT8[:, sl])
        wpk = np.empty((128, WPC), E3NP)
        wpk[:, :49 * 512] = np.ascontiguousarray(
            shard[:, :49 * 128].reshape(DC, 128, 49, 128)
            .transpose(1, 2, 0, 3).reshape(128, 49 * 512))
        wpk[:, 49 * 512:] = np.ascontiguousarray(
            shard[:, 49 * 128:].reshape(DC, 128, LB)
            .transpose(1, 0, 2).reshape(128, DC * LB))
        m["wp"] = wpk
        mk = smk.copy()
        # bp block tile: [p, j] = bp[block j, row p] * WSC
        mk[:, SM_BP:SM_BP + NBLK] = \
            bp_pad[c * NBLK * 128:(c + 1) * NBLK * 128] \
            .reshape(NBLK, 128).T.astype(np.float16)
        m["smk"] = mk
        in_maps.append(m)
    return in_maps


def _run(in_maps, trace=False, tmpdir=None):
    nc = _get_bass()
    return run_bass_kernel_spmd(nc, in_maps, list(range(NCORES)),
                                trace=trace, tmpdir=tmpdir)


def kernel(**inputs) -> np.ndarray:
    in_maps = _prep_inputs(inputs)
    try:
        res = _run(in_maps).results
    except Exception:
        # transient NRT device wedges have been observed on this fabric;
        # one retry after a short pause usually lands on healthy cores
        import time as _time
        _time.sleep(5.0)
        res = _run(in_maps).results
    # out[p, blk*B + b] = WSC*logits[b, c*VS + blk*128 + p]
    parts = []
    for c in range(NCORES):
        r = np.asarray(res[c]["logits"])
        full = (r[:, :49 * B].reshape(128, 49, B).transpose(2, 1, 0)
                .reshape(B, 49 * 128))
        tail = r[0:LB, 49 * B:50 * B].T            # [B, LB]
        parts.append(np.concatenate([full, tail], axis=1))
    logits = np.concatenate(parts, axis=1)[:, :V].astype(np.float32)
    logits *= (1.0 / WSC)  # exact: power-of-two exponent shift
    return np.broadcast_to(logits[:, None, :], (B, L - 1, V))
